# revision 28
# baseline (speedup 1.0000x reference)
"""Trainium2 Bass kernel for nn_MlpwithSOMModule (retrieval_knn).

Reference computation, per (b, k) pair with L=128, D=768:
    ctx, ent = context[b,k,0], context[b,k,1]          # [L, D] each
    S        = ctx @ ent.T                             # [L, L]
    idx      = argmax_m S[l, m]
    best     = ent[idx]                                # [L, D]
    out[l]   = f(ctx[l]) + f(best[l])                  # f = 3-layer MLP -> scalar
Gather resolved as a one-hot weighted sum of scalars:
    out[l] = f(ctx[l]) + sum_m onehot[l,m] * f(ent[m]),  onehot = (S == rowmax(S))

All matmuls contract over D, so activations live in transposed layout
[d_partition, row_free]; raw inputs are transposed once on the PE and every
later layer produces its output already transposed.

Precision: everything fp16 (1 cy/row on the PE vs 4 for fp32).  Scores use
fp16 operands with exact fp32 PSUM accumulation; on the staged inputs this
flips 18 of 32768 argmax rows (rel_l2 1.1e-2 < 2e-2 gate) and the min fp16
top-2 score gap is 1.07e-4 -- far above HW summation-order noise, so the HW
flip count matches the numpy simulation (HW-verified: 1.106e-2, identical).
MLP in fp16 contributes 4.4e-4.

The third MLP layer (768->1, w3) is folded away entirely:
  - |w3_j| becomes the ACT scale of the L2 ReLU evacuation
    (ap_j = relu(z2*|w3_j| + |w3_j|*b2_j) = |w3_j| * relu(z2 + b2_j)),
  - sign(w3_j) rides the DVE sign-chain a = sum_j s_j*ap_j (fused
    scalar_tensor_tensor), and one ones.T @ a matmul both reduces over
    partitions and broadcasts f(row) to all 128 partitions,
  - b3 is added by the tail's fused scalar_tensor_tensor.

Transposes run in fp16 (raw tiles are pre-cast; fp16 LDWEIGHTS is 97ns vs
fp32's ~140 and the transpose cadence is LDWEIGHTS-bound, measured ~73ns
vs 125), grouped 4-per-PSUM-tile with one wide 2-byte evacuation per chunk
alternating DVE/ACT.  Scheduling is FIFO-aware throughout: weight casts
and pre-casts are emitted so a DMA wait can never sit ahead of a hot
evacuation in an engine queue (head-of-line blocking there showed up as
0.7-5.6us/iteration of PE stall), and the tail is deferred one extra
pipeline stage so its obc dependency is resolved before it reaches the
DVE queue head.  HW: 319us vs the 451us f32r baseline (PE ~93% busy,
MLP matmuls at the 213ns N=512 streaming floor).

Sharding: data-parallel over the 256 (b,k) pairs -> 32 per NeuronCore,
weights replicated.  Two pairs per inner iteration (moving dim 512).
"""

from contextlib import ExitStack

import numpy as np

import concourse.bacc as bacc
import concourse.mybir as mybir
import concourse.tile as tile
from concourse.bass_utils import run_bass_kernel_spmd
from concourse.masks import make_identity

B, K, L, D = 4, 64, 128, 768
N_CORES = 8
BK = B * K                      # 256 (b,k) pairs total
BK_PER_CORE = BK // N_CORES     # 32
PAIR = 2                        # pairs per inner iteration (moving dim 512)
DC = D // 128                   # 6 contraction chunks
NCOL = PAIR * 2 * 128           # 512 columns per iteration

F32 = mybir.dt.float32
F16 = mybir.dt.float16


def build_kernel(n_bk: int = BK_PER_CORE):
    assert n_bk % PAIR == 0
    nc = bacc.Bacc("TRN2", target_bir_lowering=False)

    x = nc.declare_dram_parameter("x", [n_bk, 2, L, D], F32, isOutput=False)
    w1 = nc.declare_dram_parameter("w1", [D, D], F32, isOutput=False)
    b1 = nc.declare_dram_parameter("b1", [D], F32, isOutput=False)
    w2 = nc.declare_dram_parameter("w2", [D, D], F32, isOutput=False)
    b2 = nc.declare_dram_parameter("b2", [D], F32, isOutput=False)
    w3 = nc.declare_dram_parameter("w3", [D, 1], F32, isOutput=False)
    b3 = nc.declare_dram_parameter("b3", [1], F32, isOutput=False)
    out = nc.declare_dram_parameter("out", [n_bk, L], F32, isOutput=True)

    with tile.TileContext(nc) as tc:
        with ExitStack() as ctx:
            _emit(ctx, tc, n_bk, x, w1, b1, w2, b2, w3, b3, out)
    nc.compile()
    return nc


def _emit(ctx, tc, n_bk, x, w1, b1, w2, b2, w3, b3, out):
    nc = tc.nc
    AF = mybir.ActivationFunctionType
    ALU = mybir.AluOpType

    consts = ctx.enter_context(tc.tile_pool(name="consts", bufs=1))
    raw = ctx.enter_context(tc.tile_pool(name="raw", bufs=1))
    xt = ctx.enter_context(tc.tile_pool(name="xt", bufs=3))
    hp = ctx.enter_context(tc.tile_pool(name="hp", bufs=3))
    small = ctx.enter_context(tc.tile_pool(name="small", bufs=4))
    scratch = ctx.enter_context(tc.tile_pool(name="scratch", bufs=4))
    pmm = ctx.enter_context(tc.tile_pool(name="pmm", bufs=2, space="PSUM"))
    p512 = ctx.enter_context(tc.tile_pool(name="p512", bufs=4, space="PSUM"))
    p128 = ctx.enter_context(tc.tile_pool(name="p128", bufs=1, space="PSUM"))
    posm = ctx.enter_context(tc.tile_pool(name="posm", bufs=1, space="PSUM"))

    # ---- constant tiles (DMAs emitted later, after the first raw loads) ----
    b1_sb = consts.tile([128, DC], F32)
    b2_sb = consts.tile([128, DC], F32)
    b3_sb = consts.tile([1, 1], F32)
    w3_sb = consts.tile([128, DC], F32)
    w1h = consts.tile([128, DC, D], F16)
    w2h = consts.tile([128, DC, D], F16)
    wtmp1 = consts.tile([128, DC, D], F32)
    wtmp2 = consts.tile([128, DC, D], F32)
    w3a_sb = consts.tile([128, DC], F32)       # |w3|
    b2w_sb = consts.tile([128, DC], F32)       # b2 * |w3|
    s_sb = consts.tile([128, DC], F32)         # sign(w3) in {+1,-1}
    ones_h = consts.tile([128, 128], F16)
    sgn_h = consts.tile([128, DC, 128], F16)   # sign(w3) broadcast along free
    zeros_sb = consts.tile([128, 128], F32)
    b3x2 = consts.tile([128, 1], F32)          # 2*b3 on every partition

    ident = consts.tile([128, 128], F32)
    make_identity(nc, ident)
    ident_h = consts.tile([128, 128], F16)
    nc.vector.tensor_copy(ident_h, ident)
    oh2 = [consts.tile([128, 256], F32, name=f"oh2_{i}") for i in range(4)]
    for t in oh2:
        nc.vector.tensor_copy(t[:, 0:128], ident)

    res_all = consts.tile([128, n_bk], F32)

    def emit_const_loads():
        # small consts; emitted after the first raw-tile DMAs so iteration
        # 0's transposes aren't queued behind them
        nc.sync.dma_start(out=b1_sb, in_=b1.rearrange("(c p) -> p c", p=128))
        nc.sync.dma_start(out=b2_sb, in_=b2.rearrange("(c p) -> p c", p=128))
        nc.sync.dma_start(out=b3_sb, in_=b3[:].unsqueeze(0))
        nc.sync.dma_start(out=w3_sb, in_=w3.rearrange("(c p) one -> p (c one)", p=128))
        nc.gpsimd.partition_broadcast(b3x2, b3_sb, channels=128)
        nc.vector.tensor_scalar_mul(b3x2, b3x2, 2.0)
        # |w3|, b2*|w3|, sign(w3), and the sign broadcast tiles
        nc.vector.scalar_tensor_tensor(
            out=w3a_sb, in0=w3_sb, scalar=-1.0, in1=w3_sb, op0=ALU.mult, op1=ALU.max
        )
        nc.vector.tensor_mul(b2w_sb, b2_sb, w3a_sb)
        nc.vector.tensor_scalar(
            out=s_sb, in0=w3_sb, scalar1=0.0, scalar2=None, op0=ALU.is_ge
        )
        nc.vector.tensor_scalar(
            out=s_sb, in0=s_sb, scalar1=2.0, scalar2=-1.0, op0=ALU.mult, op1=ALU.add
        )
        nc.vector.memset(ones_h, 1.0)

    def emit_weight_load(wsrc, wtmp, wh, engine):
        # chunked fp32 stage + fp16 cast, so the first MLP layer can start
        # as soon as its first contraction chunk has landed
        for c in range(DC):
            nc.sync.dma_start(
                out=wtmp[:, c, :],
                in_=wsrc.rearrange("(c p) j -> p c j", p=128)[:, c, :],
            )
            if engine == "act":
                nc.scalar.activation(out=wh[:, c, :], in_=wtmp[:, c, :], func=AF.Copy)
            else:
                nc.vector.tensor_copy(wh[:, c, :], wtmp[:, c, :])

    n_iter = n_bk // PAIR

    def emit_load(it):
        # one tile per (pair, which) so each transpose chain only waits on
        # its own slice of the DMA traffic
        tiles = []
        for q in range(PAIR * 2):
            rq = raw.tile([128, D], F32, tag="raw", bufs=3 * PAIR * 2, name=f"raw_{it}_{q}")
            nc.sync.dma_start(
                out=rq, in_=x[it * PAIR + q // 2, q % 2]
            )
            tiles.append(rq)
        return tiles

    def emit_casts(it, raw_t):
        # fp16 copies of the raw tiles: fp16 transposes load their
        # stationary in 97ns vs fp32's ~140 (the transpose cadence is
        # LDWEIGHTS-bound).  Emitted FIFO-late so the DMA wait of a fresh
        # tile can never sit ahead of hot evacuations.
        tiles = []
        for q in range(PAIR * 2):
            rh = raw.tile([128, D], F16, tag="rawh", bufs=3 * PAIR * 2, name=f"rawh_{it}_{q}")
            if it == 0:
                # startup: ACT is idle while DVE casts w1
                nc.scalar.activation(out=rh, in_=raw_t[q], func=AF.Copy)
            else:
                nc.vector.tensor_copy(rh, raw_t[q])
            tiles.append(rh)
        return tiles

    def emit_transposes(it, raw_t):
        # XT: [d_part, chunk, col] fp16.  Per chunk: 4 transposes into one
        # [128, 512] PSUM tile, then a single wide cast-evacuation,
        # alternating DVE/ACT so neither engine throttles the PE.
        xt_t = xt.tile([128, DC, NCOL], F16, tag="xt", name=f"xt_{it}")
        for c in range(DC):
            tr_ps = p512.tile([128, NCOL], F16, tag="p512", name=f"tr_{it}_{c}")
            for q in range(PAIR * 2):
                nc.tensor.transpose(
                    tr_ps[:, q * 128 : (q + 1) * 128],
                    raw_t[q][:, c * 128 : (c + 1) * 128],
                    ident_h,
                )
            if it == 0 or c % 2 == 0:
                nc.vector.tensor_copy(xt_t[:, c, :], tr_ps)
            else:
                nc.scalar.activation(out=xt_t[:, c, :], in_=tr_ps, func=AF.Copy)
        return xt_t

    def emit_scores(it, xt_t):
        # scores + one-hot per pair; fp16 operands, exact fp32 accumulation
        onehots = []
        sall = p128.tile([128, PAIR * 128], F32, tag="p128", name=f"s_{it}")
        for p in range(PAIR):
            s_ps = sall[:, p * 128 : (p + 1) * 128]
            for c in range(DC):
                nc.tensor.matmul(
                    s_ps,
                    lhsT=xt_t[:, c, (2 * p) * 128 : (2 * p + 1) * 128],
                    rhs=xt_t[:, c, (2 * p + 1) * 128 : (2 * p + 2) * 128],
                    start=(c == 0),
                    stop=(c == DC - 1),
                )
            rm = small.tile([128, 1], F32, tag="rm", name=f"rm_{it}_{p}")
            nc.vector.reduce_max(rm, s_ps, axis=mybir.AxisListType.X)
            oh = oh2[(it * PAIR + p) % 4]
            nc.vector.tensor_scalar(
                out=oh[:, 128:256], in0=s_ps, scalar1=rm, scalar2=None,
                op0=ALU.is_equal,
            )
            onehots.append(oh)
        return onehots

    def emit_l1(it, src_t):
        # dst[j, col] = relu(sum_c W1[c,j].T @ src[c] + b1_j)
        dst_t = hp.tile([128, DC, NCOL], F16, tag="h", name=f"h_l1_{it}")
        for j in range(DC):
            mm = pmm.tile([128, NCOL], F32, tag="mm", name=f"mm_l1_{it}_{j}")
            for c in range(DC):
                nc.tensor.matmul(
                    mm,
                    lhsT=w1h[:, c, j * 128 : (j + 1) * 128],
                    rhs=src_t[:, c, :],
                    start=(c == 0),
                    stop=(c == DC - 1),
                )
            nc.scalar.activation(
                out=dst_t[:, j, :], in_=mm, func=AF.Relu, bias=b1_sb[:, j : j + 1]
            )
        return dst_t

    def emit_l2(it, src_t):
        # ap[j, col] = |w3_j| * relu(sum_c W2[c,j].T @ src[c] + b2_j)
        # (the |w3| scale rides the ACT evacuation for free)
        ap_t = hp.tile([128, DC, NCOL], F16, tag="h", name=f"h_l2_{it}")
        for j in range(DC):
            mm = pmm.tile([128, NCOL], F32, tag="mm", name=f"mm_l2_{it}_{j}")
            for c in range(DC):
                nc.tensor.matmul(
                    mm,
                    lhsT=w2h[:, c, j * 128 : (j + 1) * 128],
                    rhs=src_t[:, c, :],
                    start=(c == 0),
                    stop=(c == DC - 1),
                )
            nc.scalar.activation(
                out=ap_t[:, j, :], in_=mm, func=AF.Relu,
                bias=b2w_sb[:, j : j + 1], scale=w3a_sb[:, j : j + 1],
            )
        return ap_t

    def emit_achain(it, ap_t, f16_chain=False):
        # a[p, col] = sum_j sign(w3_j)[p] * ap_j[p, col] on DVE (fused
        # mult-accumulate); final step writes fp16 for the PE.  f16_chain
        # runs the intermediates in fp16 (2x DVE rate) -- used only for the
        # epilogue where the chain is exposed serially.
        idt = F16 if f16_chain else F32
        acc = scratch.tile([128, NCOL], idt, tag="af", bufs=2, name=f"a_{it}_0")
        nc.vector.tensor_scalar(
            out=acc, in0=ap_t[:, 0, :], scalar1=s_sb[:, 0:1], scalar2=None,
            op0=ALU.mult,
        )
        for c in range(1, DC):
            nxt = (
                scratch.tile([128, NCOL], F16, tag="ah", bufs=2, name=f"ah_{it}")
                if c == DC - 1
                else scratch.tile([128, NCOL], idt, tag="af", bufs=2, name=f"a_{it}_{c}")
            )
            nc.vector.scalar_tensor_tensor(
                out=nxt, in0=ap_t[:, c, :], scalar=s_sb[:, c : c + 1],
                in1=acc, op0=ALU.mult, op1=ALU.add,
            )
            acc = nxt
        return acc

    def emit_obc(it, a_h):
        # obc[l, col] = sum_p a[p, col] = f(col-row) - b3, reduced over
        # partitions AND broadcast to all 128 by the all-ones stationary
        obc = posm.tile([128, NCOL], F32, tag="obc", name=f"obc_{it}")
        nc.tensor.matmul(obc, lhsT=ones_h, rhs=a_h, start=True, stop=True)
        return obc

    def emit_tail(it, obc, onehots):
        # res[l] = obc[l, ctx_l] + sum_m onehot[l,m] * obc[l, ent_m] + 2*b3.
        # ctx and ent columns are adjacent in obc, and the mask tile is
        # [ident | onehot], so one fused mult + free-dim accumulate yields
        # rctx + rent at once.
        for p in range(PAIR):
            prod = scratch.tile([128, 256], F32, tag="prod", name=f"prod_{it}_{p}")
            rsum = small.tile([128, 1], F32, tag="rent", name=f"rsum_{it}_{p}")
            nc.vector.scalar_tensor_tensor(
                out=prod, in0=onehots[p], scalar=1.0,
                in1=obc[:, (2 * p) * 128 : (2 * p + 2) * 128],
                op0=ALU.mult, op1=ALU.mult, accum_out=rsum,
            )
            nc.vector.tensor_scalar(
                out=res_all[:, it * PAIR + p : it * PAIR + p + 1],
                in0=rsum, scalar1=b3x2, scalar2=None, op0=ALU.add,
            )

    # Two-stage software pipeline over iterations: stage A(i) = load/transpose/
    # scores/L1, stage B(i) = L2/obc/tail.  B(i-1) pieces are interleaved
    # into A(i) so the PE always has independent work while evacuations and
    # the DVE tail of the previous iteration drain.
    state = {}
    prev = None
    prev2 = None
    raw_next = emit_load(0)
    emit_const_loads()
    emit_weight_load(w1, wtmp1, w1h, "dve")
    rawh_next = emit_casts(0, raw_next)
    raw_next = emit_load(1)
    for it in range(n_iter):
        raw_t = rawh_next
        raw_cast_src = raw_next
        if it + 2 < n_iter:
            raw_next = emit_load(it + 2)
        if prev is not None:
            state[prev]["ap"] = emit_l2(prev, state[prev]["h1"])
        xt_t = emit_transposes(it, raw_t)
        # tail(i-2) here: its obc dependency resolved long ago, so it can't
        # head-of-line-block the DVE queue ahead of the transpose evacs
        if prev2 is not None:
            emit_tail(prev2, state[prev2]["obc"], state[prev2]["oh"])
            del state[prev2]
        if prev is not None:
            state[prev]["ah"] = emit_achain(prev, state[prev]["ap"])
        onehots = emit_scores(it, xt_t)
        if it == 0:
            emit_weight_load(w2, wtmp2, w2h, "dve")
        h1 = emit_l1(it, xt_t)
        if prev is not None:
            state[prev]["obc"] = emit_obc(prev, state[prev]["ah"])
        if it + 1 < n_iter:
            rawh_next = emit_casts(it + 1, raw_cast_src)
        if it == 1:
            # sign-broadcast tiles for the epilogue obc; built here (not at
            # const-load time) so they never sit ahead of startup-critical
            # work in the DVE queue
            nc.vector.memset(zeros_sb, 0.0)
            for c in range(DC):
                nc.vector.tensor_scalar(
                    out=sgn_h[:, c, :], in0=zeros_sb, scalar1=s_sb[:, c : c + 1],
                    scalar2=None, op0=ALU.add,
                )
        state[it] = {"h1": h1, "oh": onehots}
        prev2 = prev
        prev = it
    # epilogue for the last two iterations
    if prev2 is not None:
        emit_tail(prev2, state[prev2]["obc"], state[prev2]["oh"])
        del state[prev2]
    ap_t = emit_l2(prev, state[prev]["h1"])
    obc = posm.tile([128, NCOL], F32, tag="obc", name="obc_epi")
    for j in range(DC):
        nc.tensor.matmul(
            obc, lhsT=sgn_h[:, j, :], rhs=ap_t[:, j, :],
            start=(j == 0), stop=(j == DC - 1),
        )
    emit_tail(prev, obc, state[prev]["oh"])

    # ---- store: transpose res_all [l_part, bk] on PE, contiguous DMA out ----
    res_ps = posm.tile([n_bk, 128], F32, tag="obc", name="res_ps")
    nc.tensor.transpose(res_ps, res_all, ident)
    res_T = small.tile([n_bk, 128], F32, tag="resT", name="res_T")
    nc.vector.tensor_copy(res_T, res_ps)
    nc.sync.dma_start(out=out[:, :], in_=res_T)


_NC_CACHE = {}


def _get_nc(n_bk):
    if n_bk not in _NC_CACHE:
        _NC_CACHE[n_bk] = build_kernel(n_bk)
    return _NC_CACHE[n_bk]


def run(inputs, trace=False):
    context = np.ascontiguousarray(np.asarray(inputs["context"], dtype=np.float32))
    xs = context.reshape(BK, 2, L, D)
    shared = {
        "w1": np.ascontiguousarray(np.asarray(inputs["W1"], dtype=np.float32)),
        "b1": np.ascontiguousarray(np.asarray(inputs["b1"], dtype=np.float32)),
        "w2": np.ascontiguousarray(np.asarray(inputs["W2"], dtype=np.float32)),
        "b2": np.ascontiguousarray(np.asarray(inputs["b2"], dtype=np.float32)),
        "w3": np.ascontiguousarray(np.asarray(inputs["W3"], dtype=np.float32)),
        "b3": np.ascontiguousarray(np.asarray(inputs["b3"], dtype=np.float32)),
    }
    in_maps = [
        {"x": np.ascontiguousarray(xs[c * BK_PER_CORE : (c + 1) * BK_PER_CORE]), **shared}
        for c in range(N_CORES)
    ]
    nc = _get_nc(BK_PER_CORE)
    res = run_bass_kernel_spmd(nc, in_maps, list(range(N_CORES)), trace=trace)
    outs = [m["out"] for m in res.results]
    full = np.concatenate(outs, axis=0).reshape(B, K, L).astype(np.float32)
    return full, res


def kernel(**inputs) -> np.ndarray:
    full, _ = run(inputs, trace=False)
    return full


# revision 29
# speedup vs baseline: 1.0031x; 1.0031x over previous
"""Trainium2 Bass kernel for nn_MlpwithSOMModule (retrieval_knn).

Reference computation, per (b, k) pair with L=128, D=768:
    ctx, ent = context[b,k,0], context[b,k,1]          # [L, D] each
    S        = ctx @ ent.T                             # [L, L]
    idx      = argmax_m S[l, m]
    best     = ent[idx]                                # [L, D]
    out[l]   = f(ctx[l]) + f(best[l])                  # f = 3-layer MLP -> scalar
Gather resolved as a one-hot weighted sum of scalars:
    out[l] = f(ctx[l]) + sum_m onehot[l,m] * f(ent[m]),  onehot = (S == rowmax(S))

All matmuls contract over D, so activations live in transposed layout
[d_partition, row_free]; raw inputs are transposed once on the PE and every
later layer produces its output already transposed.

Precision: everything fp16 (1 cy/row on the PE vs 4 for fp32).  Scores use
fp16 operands with exact fp32 PSUM accumulation; on the staged inputs this
flips 18 of 32768 argmax rows (rel_l2 1.1e-2 < 2e-2 gate) and the min fp16
top-2 score gap is 1.07e-4 -- far above HW summation-order noise, so the HW
flip count matches the numpy simulation (HW-verified: 1.106e-2, identical).
MLP in fp16 contributes 4.4e-4.

The third MLP layer (768->1, w3) is folded away entirely:
  - |w3_j| becomes the ACT scale of the L2 ReLU evacuation
    (ap_j = relu(z2*|w3_j| + |w3_j|*b2_j) = |w3_j| * relu(z2 + b2_j)),
  - sign(w3_j) rides the DVE sign-chain a = sum_j s_j*ap_j (fused
    scalar_tensor_tensor), and one ones.T @ a matmul both reduces over
    partitions and broadcasts f(row) to all 128 partitions,
  - b3 is added by the tail's fused scalar_tensor_tensor.

Transposes run in fp16 (raw tiles are pre-cast; fp16 LDWEIGHTS is 97ns vs
fp32's ~140 and the transpose cadence is LDWEIGHTS-bound, measured ~73ns
vs 125), grouped 4-per-PSUM-tile with one wide 2-byte evacuation per chunk
alternating DVE/ACT.  Scheduling is FIFO-aware throughout: weight casts
and pre-casts are emitted so a DMA wait can never sit ahead of a hot
evacuation in an engine queue (head-of-line blocking there showed up as
0.7-5.6us/iteration of PE stall), and the tail is deferred one extra
pipeline stage so its obc dependency is resolved before it reaches the
DVE queue head.  HW: 319us vs the 451us f32r baseline (PE ~93% busy,
MLP matmuls at the 213ns N=512 streaming floor).

Sharding: data-parallel over the 256 (b,k) pairs -> 32 per NeuronCore,
weights replicated.  Two pairs per inner iteration (moving dim 512).
"""

from contextlib import ExitStack

import numpy as np

import concourse.bacc as bacc
import concourse.mybir as mybir
import concourse.tile as tile
from concourse.bass_utils import run_bass_kernel_spmd
from concourse.masks import make_identity

B, K, L, D = 4, 64, 128, 768
N_CORES = 8
BK = B * K                      # 256 (b,k) pairs total
BK_PER_CORE = BK // N_CORES     # 32
PAIR = 2                        # pairs per inner iteration (moving dim 512)
DC = D // 128                   # 6 contraction chunks
NCOL = PAIR * 2 * 128           # 512 columns per iteration

F32 = mybir.dt.float32
F16 = mybir.dt.float16


def build_kernel(n_bk: int = BK_PER_CORE):
    assert n_bk % PAIR == 0
    nc = bacc.Bacc("TRN2", target_bir_lowering=False)

    x = nc.declare_dram_parameter("x", [n_bk, 2, L, D], F32, isOutput=False)
    w1 = nc.declare_dram_parameter("w1", [D, D], F32, isOutput=False)
    b1 = nc.declare_dram_parameter("b1", [D], F32, isOutput=False)
    w2 = nc.declare_dram_parameter("w2", [D, D], F32, isOutput=False)
    b2 = nc.declare_dram_parameter("b2", [D], F32, isOutput=False)
    w3 = nc.declare_dram_parameter("w3", [D, 1], F32, isOutput=False)
    b3 = nc.declare_dram_parameter("b3", [1], F32, isOutput=False)
    out = nc.declare_dram_parameter("out", [n_bk, L], F32, isOutput=True)

    with tile.TileContext(nc) as tc:
        with ExitStack() as ctx:
            _emit(ctx, tc, n_bk, x, w1, b1, w2, b2, w3, b3, out)
    nc.compile()
    return nc


def _emit(ctx, tc, n_bk, x, w1, b1, w2, b2, w3, b3, out):
    nc = tc.nc
    AF = mybir.ActivationFunctionType
    ALU = mybir.AluOpType

    consts = ctx.enter_context(tc.tile_pool(name="consts", bufs=1))
    raw = ctx.enter_context(tc.tile_pool(name="raw", bufs=1))
    xt = ctx.enter_context(tc.tile_pool(name="xt", bufs=3))
    hp = ctx.enter_context(tc.tile_pool(name="hp", bufs=3))
    small = ctx.enter_context(tc.tile_pool(name="small", bufs=4))
    scratch = ctx.enter_context(tc.tile_pool(name="scratch", bufs=4))
    pmm = ctx.enter_context(tc.tile_pool(name="pmm", bufs=2, space="PSUM"))
    p512 = ctx.enter_context(tc.tile_pool(name="p512", bufs=4, space="PSUM"))
    p128 = ctx.enter_context(tc.tile_pool(name="p128", bufs=1, space="PSUM"))
    posm = ctx.enter_context(tc.tile_pool(name="posm", bufs=1, space="PSUM"))

    # ---- constant tiles (DMAs emitted later, after the first raw loads) ----
    b1_sb = consts.tile([128, DC], F32)
    b2_sb = consts.tile([128, DC], F32)
    b3_sb = consts.tile([1, 1], F32)
    w3_sb = consts.tile([128, DC], F32)
    w1h = consts.tile([128, DC, D], F16)
    w2h = consts.tile([128, DC, D], F16)
    wtmp1 = consts.tile([128, DC, D], F32)
    wtmp2 = consts.tile([128, DC, D], F32)
    w3a_sb = consts.tile([128, DC], F32)       # |w3|
    b2w_sb = consts.tile([128, DC], F32)       # b2 * |w3|
    s_sb = consts.tile([128, DC], F32)         # sign(w3) in {+1,-1}
    ones_h = consts.tile([128, 128], F16)
    sgn_h = consts.tile([128, DC, 128], F16)   # sign(w3) broadcast along free
    zeros_sb = consts.tile([128, 128], F32)
    b3x2 = consts.tile([128, 1], F32)          # 2*b3 on every partition

    ident = consts.tile([128, 128], F32)
    make_identity(nc, ident)
    ident_h = consts.tile([128, 128], F16)
    nc.vector.tensor_copy(ident_h, ident)
    oh2 = [consts.tile([128, 256], F32, name=f"oh2_{i}") for i in range(4)]
    for t in oh2:
        nc.vector.tensor_copy(t[:, 0:128], ident)

    res_all = consts.tile([128, n_bk], F32)

    def emit_const_loads():
        # small consts; emitted after the first raw-tile DMAs so iteration
        # 0's transposes aren't queued behind them
        nc.sync.dma_start(out=b1_sb, in_=b1.rearrange("(c p) -> p c", p=128))
        nc.sync.dma_start(out=b2_sb, in_=b2.rearrange("(c p) -> p c", p=128))
        nc.sync.dma_start(out=b3_sb, in_=b3[:].unsqueeze(0))
        nc.sync.dma_start(out=w3_sb, in_=w3.rearrange("(c p) one -> p (c one)", p=128))
        nc.gpsimd.partition_broadcast(b3x2, b3_sb, channels=128)
        nc.vector.tensor_scalar_mul(b3x2, b3x2, 2.0)
        # |w3|, b2*|w3|, sign(w3), and the sign broadcast tiles
        nc.vector.scalar_tensor_tensor(
            out=w3a_sb, in0=w3_sb, scalar=-1.0, in1=w3_sb, op0=ALU.mult, op1=ALU.max
        )
        nc.vector.tensor_mul(b2w_sb, b2_sb, w3a_sb)
        nc.vector.tensor_scalar(
            out=s_sb, in0=w3_sb, scalar1=0.0, scalar2=None, op0=ALU.is_ge
        )
        nc.vector.tensor_scalar(
            out=s_sb, in0=s_sb, scalar1=2.0, scalar2=-1.0, op0=ALU.mult, op1=ALU.add
        )
        nc.vector.memset(ones_h, 1.0)

    def emit_weight_load(wsrc, wtmp, wh, engine):
        # chunked fp32 stage + fp16 cast, so the first MLP layer can start
        # as soon as its first contraction chunk has landed
        for c in range(DC):
            nc.sync.dma_start(
                out=wtmp[:, c, :],
                in_=wsrc.rearrange("(c p) j -> p c j", p=128)[:, c, :],
            )
            if engine == "act":
                nc.scalar.activation(out=wh[:, c, :], in_=wtmp[:, c, :], func=AF.Copy)
            else:
                nc.vector.tensor_copy(wh[:, c, :], wtmp[:, c, :])

    n_iter = n_bk // PAIR

    def emit_load(it):
        # one tile per (pair, which) so each transpose chain only waits on
        # its own slice of the DMA traffic
        tiles = []
        for q in range(PAIR * 2):
            rq = raw.tile([128, D], F32, tag="raw", bufs=3 * PAIR * 2, name=f"raw_{it}_{q}")
            nc.sync.dma_start(
                out=rq, in_=x[it * PAIR + q // 2, q % 2]
            )
            tiles.append(rq)
        return tiles

    def emit_casts(it, raw_t):
        # fp16 copies of the raw tiles: fp16 transposes load their
        # stationary in 97ns vs fp32's ~140 (the transpose cadence is
        # LDWEIGHTS-bound).  Emitted FIFO-late so the DMA wait of a fresh
        # tile can never sit ahead of hot evacuations.
        tiles = []
        for q in range(PAIR * 2):
            rh = raw.tile([128, D], F16, tag="rawh", bufs=3 * PAIR * 2, name=f"rawh_{it}_{q}")
            if it == 0:
                # startup: ACT is idle while DVE casts w1
                nc.scalar.activation(out=rh, in_=raw_t[q], func=AF.Copy)
            else:
                nc.vector.tensor_copy(rh, raw_t[q])
            tiles.append(rh)
        return tiles

    def emit_transposes(it, raw_t):
        # XT: [d_part, chunk, col] fp16.  Per chunk: 4 transposes into one
        # [128, 512] PSUM tile, then a single wide cast-evacuation,
        # alternating DVE/ACT so neither engine throttles the PE.
        xt_t = xt.tile([128, DC, NCOL], F16, tag="xt", name=f"xt_{it}")
        for c in range(DC):
            tr_ps = p512.tile([128, NCOL], F16, tag="p512", name=f"tr_{it}_{c}")
            for q in range(PAIR * 2):
                nc.tensor.transpose(
                    tr_ps[:, q * 128 : (q + 1) * 128],
                    raw_t[q][:, c * 128 : (c + 1) * 128],
                    ident_h,
                )
            if it == 0 or c % 2 == 0:
                nc.vector.tensor_copy(xt_t[:, c, :], tr_ps)
            else:
                nc.scalar.activation(out=xt_t[:, c, :], in_=tr_ps, func=AF.Copy)
        return xt_t

    def emit_scores(it, xt_t):
        # scores + one-hot per pair; fp16 operands, exact fp32 accumulation
        onehots = []
        sall = p128.tile([128, PAIR * 128], F32, tag="p128", name=f"s_{it}")
        for p in range(PAIR):
            s_ps = sall[:, p * 128 : (p + 1) * 128]
            for c in range(DC):
                nc.tensor.matmul(
                    s_ps,
                    lhsT=xt_t[:, c, (2 * p) * 128 : (2 * p + 1) * 128],
                    rhs=xt_t[:, c, (2 * p + 1) * 128 : (2 * p + 2) * 128],
                    start=(c == 0),
                    stop=(c == DC - 1),
                )
            rm = small.tile([128, 1], F32, tag="rm", name=f"rm_{it}_{p}")
            nc.vector.reduce_max(rm, s_ps, axis=mybir.AxisListType.X)
            oh = oh2[(it * PAIR + p) % 4]
            nc.vector.tensor_scalar(
                out=oh[:, 128:256], in0=s_ps, scalar1=rm, scalar2=None,
                op0=ALU.is_equal,
            )
            onehots.append(oh)
        return onehots

    def emit_l1(it, src_t):
        # dst[j, col] = relu(sum_c W1[c,j].T @ src[c] + b1_j)
        dst_t = hp.tile([128, DC, NCOL], F16, tag="h", name=f"h_l1_{it}")
        for j in range(DC):
            mm = pmm.tile([128, NCOL], F32, tag="mm", name=f"mm_l1_{it}_{j}")
            for c in range(DC):
                nc.tensor.matmul(
                    mm,
                    lhsT=w1h[:, c, j * 128 : (j + 1) * 128],
                    rhs=src_t[:, c, :],
                    start=(c == 0),
                    stop=(c == DC - 1),
                )
            nc.scalar.activation(
                out=dst_t[:, j, :], in_=mm, func=AF.Relu, bias=b1_sb[:, j : j + 1]
            )
        return dst_t

    def emit_l2(it, src_t):
        # ap[j, col] = |w3_j| * relu(sum_c W2[c,j].T @ src[c] + b2_j)
        # (the |w3| scale rides the ACT evacuation for free)
        ap_t = hp.tile([128, DC, NCOL], F16, tag="h", name=f"h_l2_{it}")
        for j in range(DC):
            mm = pmm.tile([128, NCOL], F32, tag="mm", name=f"mm_l2_{it}_{j}")
            for c in range(DC):
                nc.tensor.matmul(
                    mm,
                    lhsT=w2h[:, c, j * 128 : (j + 1) * 128],
                    rhs=src_t[:, c, :],
                    start=(c == 0),
                    stop=(c == DC - 1),
                )
            nc.scalar.activation(
                out=ap_t[:, j, :], in_=mm, func=AF.Relu,
                bias=b2w_sb[:, j : j + 1], scale=w3a_sb[:, j : j + 1],
            )
        return ap_t

    def emit_achain(it, ap_t, f16_chain=False):
        # a[p, col] = sum_j sign(w3_j)[p] * ap_j[p, col] on DVE (fused
        # mult-accumulate); final step writes fp16 for the PE.  f16_chain
        # runs the intermediates in fp16 (2x DVE rate) -- used only for the
        # epilogue where the chain is exposed serially.
        idt = F16 if f16_chain else F32
        acc = scratch.tile([128, NCOL], idt, tag="af", bufs=2, name=f"a_{it}_0")
        nc.vector.tensor_scalar(
            out=acc, in0=ap_t[:, 0, :], scalar1=s_sb[:, 0:1], scalar2=None,
            op0=ALU.mult,
        )
        for c in range(1, DC):
            nxt = (
                scratch.tile([128, NCOL], F16, tag="ah", bufs=2, name=f"ah_{it}")
                if c == DC - 1
                else scratch.tile([128, NCOL], idt, tag="af", bufs=2, name=f"a_{it}_{c}")
            )
            nc.vector.scalar_tensor_tensor(
                out=nxt, in0=ap_t[:, c, :], scalar=s_sb[:, c : c + 1],
                in1=acc, op0=ALU.mult, op1=ALU.add,
            )
            acc = nxt
        return acc

    def emit_obc(it, a_h):
        # obc[l, col] = sum_p a[p, col] = f(col-row) - b3, reduced over
        # partitions AND broadcast to all 128 by the all-ones stationary
        obc = posm.tile([128, NCOL], F32, tag="obc", name=f"obc_{it}")
        nc.tensor.matmul(obc, lhsT=ones_h, rhs=a_h, start=True, stop=True)
        return obc

    def emit_tail(it, obc, onehots):
        # res[l] = obc[l, ctx_l] + sum_m onehot[l,m] * obc[l, ent_m] + 2*b3.
        # ctx and ent columns are adjacent in obc, and the mask tile is
        # [ident | onehot], so one fused mult + free-dim accumulate yields
        # rctx + rent at once.
        for p in range(PAIR):
            prod = scratch.tile([128, 256], F32, tag="prod", name=f"prod_{it}_{p}")
            rsum = small.tile([128, 1], F32, tag="rent", name=f"rsum_{it}_{p}")
            nc.vector.scalar_tensor_tensor(
                out=prod, in0=onehots[p], scalar=1.0,
                in1=obc[:, (2 * p) * 128 : (2 * p + 2) * 128],
                op0=ALU.mult, op1=ALU.mult, accum_out=rsum,
            )
            nc.vector.tensor_scalar(
                out=res_all[:, it * PAIR + p : it * PAIR + p + 1],
                in0=rsum, scalar1=b3x2, scalar2=None, op0=ALU.add,
            )

    # Two-stage software pipeline over iterations: stage A(i) = load/transpose/
    # scores/L1, stage B(i) = L2/obc/tail.  B(i-1) pieces are interleaved
    # into A(i) so the PE always has independent work while evacuations and
    # the DVE tail of the previous iteration drain.
    state = {}
    prev = None
    prev2 = None
    raw_next = emit_load(0)
    emit_const_loads()
    emit_weight_load(w1, wtmp1, w1h, "dve")
    rawh_next = emit_casts(0, raw_next)
    for it in range(n_iter):
        raw_t = rawh_next
        if it + 1 < n_iter:
            raw_next = emit_load(it + 1)
        if prev is not None:
            state[prev]["ap"] = emit_l2(prev, state[prev]["h1"])
        xt_t = emit_transposes(it, raw_t)
        # tail(i-2) here: its obc dependency resolved long ago, so it can't
        # head-of-line-block the DVE queue ahead of the transpose evacs
        if prev2 is not None:
            emit_tail(prev2, state[prev2]["obc"], state[prev2]["oh"])
            del state[prev2]
        if prev is not None:
            state[prev]["ah"] = emit_achain(prev, state[prev]["ap"])
        onehots = emit_scores(it, xt_t)
        if it == 0:
            emit_weight_load(w2, wtmp2, w2h, "dve")
        h1 = emit_l1(it, xt_t)
        if prev is not None:
            state[prev]["obc"] = emit_obc(prev, state[prev]["ah"])
        if it + 1 < n_iter:
            rawh_next = emit_casts(it + 1, raw_next)
        if it == 1:
            # sign-broadcast tiles for the epilogue obc; built here (not at
            # const-load time) so they never sit ahead of startup-critical
            # work in the DVE queue
            nc.vector.memset(zeros_sb, 0.0)
            for c in range(DC):
                nc.vector.tensor_scalar(
                    out=sgn_h[:, c, :], in0=zeros_sb, scalar1=s_sb[:, c : c + 1],
                    scalar2=None, op0=ALU.add,
                )
        state[it] = {"h1": h1, "oh": onehots}
        prev2 = prev
        prev = it
    # epilogue for the last two iterations
    if prev2 is not None:
        emit_tail(prev2, state[prev2]["obc"], state[prev2]["oh"])
        del state[prev2]
    ap_t = emit_l2(prev, state[prev]["h1"])
    obc = posm.tile([128, NCOL], F32, tag="obc", name="obc_epi")
    for j in range(DC):
        nc.tensor.matmul(
            obc, lhsT=sgn_h[:, j, :], rhs=ap_t[:, j, :],
            start=(j == 0), stop=(j == DC - 1),
        )
    emit_tail(prev, obc, state[prev]["oh"])

    # ---- store: transpose res_all [l_part, bk] on PE, contiguous DMA out ----
    res_ps = posm.tile([n_bk, 128], F32, tag="obc", name="res_ps")
    nc.tensor.transpose(res_ps, res_all, ident)
    res_T = small.tile([n_bk, 128], F32, tag="resT", name="res_T")
    nc.vector.tensor_copy(res_T, res_ps)
    nc.sync.dma_start(out=out[:, :], in_=res_T)


_NC_CACHE = {}


def _get_nc(n_bk):
    if n_bk not in _NC_CACHE:
        _NC_CACHE[n_bk] = build_kernel(n_bk)
    return _NC_CACHE[n_bk]


def run(inputs, trace=False):
    context = np.ascontiguousarray(np.asarray(inputs["context"], dtype=np.float32))
    xs = context.reshape(BK, 2, L, D)
    shared = {
        "w1": np.ascontiguousarray(np.asarray(inputs["W1"], dtype=np.float32)),
        "b1": np.ascontiguousarray(np.asarray(inputs["b1"], dtype=np.float32)),
        "w2": np.ascontiguousarray(np.asarray(inputs["W2"], dtype=np.float32)),
        "b2": np.ascontiguousarray(np.asarray(inputs["b2"], dtype=np.float32)),
        "w3": np.ascontiguousarray(np.asarray(inputs["W3"], dtype=np.float32)),
        "b3": np.ascontiguousarray(np.asarray(inputs["b3"], dtype=np.float32)),
    }
    in_maps = [
        {"x": np.ascontiguousarray(xs[c * BK_PER_CORE : (c + 1) * BK_PER_CORE]), **shared}
        for c in range(N_CORES)
    ]
    nc = _get_nc(BK_PER_CORE)
    res = run_bass_kernel_spmd(nc, in_maps, list(range(N_CORES)), trace=trace)
    outs = [m["out"] for m in res.results]
    full = np.concatenate(outs, axis=0).reshape(B, K, L).astype(np.float32)
    return full, res


def kernel(**inputs) -> np.ndarray:
    full, _ = run(inputs, trace=False)
    return full


# revision 30
# speedup vs baseline: 1.0035x; 1.0004x over previous
"""Trainium2 Bass kernel for nn_MlpwithSOMModule (retrieval_knn).

Reference computation, per (b, k) pair with L=128, D=768:
    ctx, ent = context[b,k,0], context[b,k,1]          # [L, D] each
    S        = ctx @ ent.T                             # [L, L]
    idx      = argmax_m S[l, m]
    best     = ent[idx]                                # [L, D]
    out[l]   = f(ctx[l]) + f(best[l])                  # f = 3-layer MLP -> scalar
Gather resolved as a one-hot weighted sum of scalars:
    out[l] = f(ctx[l]) + sum_m onehot[l,m] * f(ent[m]),  onehot = (S == rowmax(S))

All matmuls contract over D, so activations live in transposed layout
[d_partition, row_free]; raw inputs are transposed once on the PE and every
later layer produces its output already transposed.

Precision: everything fp16 (1 cy/row on the PE vs 4 for fp32).  Scores use
fp16 operands with exact fp32 PSUM accumulation; on the staged inputs this
flips 18 of 32768 argmax rows (rel_l2 1.1e-2 < 2e-2 gate) and the min fp16
top-2 score gap is 1.07e-4 -- far above HW summation-order noise, so the HW
flip count matches the numpy simulation (HW-verified: 1.106e-2, identical).
MLP in fp16 contributes 4.4e-4.

The third MLP layer (768->1, w3) is folded away entirely:
  - |w3_j| becomes the ACT scale of the L2 ReLU evacuation
    (ap_j = relu(z2*|w3_j| + |w3_j|*b2_j) = |w3_j| * relu(z2 + b2_j)),
  - sign(w3_j) rides the DVE sign-chain a = sum_j s_j*ap_j (fused
    scalar_tensor_tensor), and one ones.T @ a matmul both reduces over
    partitions and broadcasts f(row) to all 128 partitions,
  - b3 is added by the tail's fused scalar_tensor_tensor.

Transposes run in fp16 (raw tiles are pre-cast; fp16 LDWEIGHTS is 97ns vs
fp32's ~140 and the transpose cadence is LDWEIGHTS-bound, measured ~73ns
vs 125), grouped 4-per-PSUM-tile with one wide 2-byte evacuation per chunk
alternating DVE/ACT.  Scheduling is FIFO-aware throughout: weight casts
and pre-casts are emitted so a DMA wait can never sit ahead of a hot
evacuation in an engine queue (head-of-line blocking there showed up as
0.7-5.6us/iteration of PE stall), and the tail is deferred one extra
pipeline stage so its obc dependency is resolved before it reaches the
DVE queue head.  HW: 319us vs the 451us f32r baseline (PE ~93% busy,
MLP matmuls at the 213ns N=512 streaming floor).

Sharding: data-parallel over the 256 (b,k) pairs -> 32 per NeuronCore,
weights replicated.  Two pairs per inner iteration (moving dim 512).
"""

from contextlib import ExitStack

import numpy as np

import concourse.bacc as bacc
import concourse.mybir as mybir
import concourse.tile as tile
from concourse.bass_utils import run_bass_kernel_spmd
from concourse.masks import make_identity

B, K, L, D = 4, 64, 128, 768
N_CORES = 8
BK = B * K                      # 256 (b,k) pairs total
BK_PER_CORE = BK // N_CORES     # 32
PAIR = 2                        # pairs per inner iteration (moving dim 512)
DC = D // 128                   # 6 contraction chunks
NCOL = PAIR * 2 * 128           # 512 columns per iteration

F32 = mybir.dt.float32
F16 = mybir.dt.float16


def build_kernel(n_bk: int = BK_PER_CORE):
    assert n_bk % PAIR == 0
    nc = bacc.Bacc("TRN2", target_bir_lowering=False)

    x = nc.declare_dram_parameter("x", [n_bk, 2, L, D], F32, isOutput=False)
    w1 = nc.declare_dram_parameter("w1", [D, D], F32, isOutput=False)
    b1 = nc.declare_dram_parameter("b1", [D], F32, isOutput=False)
    w2 = nc.declare_dram_parameter("w2", [D, D], F32, isOutput=False)
    b2 = nc.declare_dram_parameter("b2", [D], F32, isOutput=False)
    w3 = nc.declare_dram_parameter("w3", [D, 1], F32, isOutput=False)
    b3 = nc.declare_dram_parameter("b3", [1], F32, isOutput=False)
    out = nc.declare_dram_parameter("out", [n_bk, L], F32, isOutput=True)

    with tile.TileContext(nc) as tc:
        with ExitStack() as ctx:
            _emit(ctx, tc, n_bk, x, w1, b1, w2, b2, w3, b3, out)
    nc.compile()
    return nc


def _emit(ctx, tc, n_bk, x, w1, b1, w2, b2, w3, b3, out):
    nc = tc.nc
    AF = mybir.ActivationFunctionType
    ALU = mybir.AluOpType

    consts = ctx.enter_context(tc.tile_pool(name="consts", bufs=1))
    raw = ctx.enter_context(tc.tile_pool(name="raw", bufs=1))
    xt = ctx.enter_context(tc.tile_pool(name="xt", bufs=3))
    hp = ctx.enter_context(tc.tile_pool(name="hp", bufs=3))
    small = ctx.enter_context(tc.tile_pool(name="small", bufs=4))
    scratch = ctx.enter_context(tc.tile_pool(name="scratch", bufs=4))
    pmm = ctx.enter_context(tc.tile_pool(name="pmm", bufs=2, space="PSUM"))
    p512 = ctx.enter_context(tc.tile_pool(name="p512", bufs=4, space="PSUM"))
    p128 = ctx.enter_context(tc.tile_pool(name="p128", bufs=1, space="PSUM"))
    posm = ctx.enter_context(tc.tile_pool(name="posm", bufs=1, space="PSUM"))

    # ---- constant tiles (DMAs emitted later, after the first raw loads) ----
    b1_sb = consts.tile([128, DC], F32)
    b2_sb = consts.tile([128, DC], F32)
    b3_sb = consts.tile([1, 1], F32)
    w3_sb = consts.tile([128, DC], F32)
    w1h = consts.tile([128, DC, D], F16)
    w2h = consts.tile([128, DC, D], F16)
    wtmp1 = consts.tile([128, DC, D], F32)
    wtmp2 = consts.tile([128, DC, D], F32)
    w3a_sb = consts.tile([128, DC], F32)       # |w3|
    b2w_sb = consts.tile([128, DC], F32)       # b2 * |w3|
    s_sb = consts.tile([128, DC], F32)         # sign(w3) in {+1,-1}
    ones_h = consts.tile([128, 128], F16)
    sgn_h = consts.tile([128, DC, 128], F16)   # sign(w3) broadcast along free
    zeros_sb = consts.tile([128, 128], F32)
    b3x2 = consts.tile([128, 1], F32)          # 2*b3 on every partition

    ident = consts.tile([128, 128], F32)
    make_identity(nc, ident)
    ident_h = consts.tile([128, 128], F16)
    nc.vector.tensor_copy(ident_h, ident)
    # HAM warm-up: ~3.4us of dummy transposes inside the first-DMA shadow so
    # the clock gate is at 2.4GHz when the real startup transposes begin
    # (outputs are never read; the pool reuses the bank afterwards)
    warm = p512.tile([128, NCOL], F16, tag="p512", name="warm")
    for i in range(16):
        nc.tensor.transpose(
            warm[:, (i % 4) * 128 : (i % 4 + 1) * 128], ident_h, ident_h
        )
    oh2 = [consts.tile([128, 256], F32, name=f"oh2_{i}") for i in range(4)]
    for t in oh2:
        nc.vector.tensor_copy(t[:, 0:128], ident)

    res_all = consts.tile([128, n_bk], F32)

    def emit_const_loads():
        # small consts; emitted after the first raw-tile DMAs so iteration
        # 0's transposes aren't queued behind them
        nc.sync.dma_start(out=b1_sb, in_=b1.rearrange("(c p) -> p c", p=128))
        nc.sync.dma_start(out=b2_sb, in_=b2.rearrange("(c p) -> p c", p=128))
        nc.sync.dma_start(out=b3_sb, in_=b3[:].unsqueeze(0))
        nc.sync.dma_start(out=w3_sb, in_=w3.rearrange("(c p) one -> p (c one)", p=128))
        nc.gpsimd.partition_broadcast(b3x2, b3_sb, channels=128)
        nc.vector.tensor_scalar_mul(b3x2, b3x2, 2.0)
        # |w3|, b2*|w3|, sign(w3), and the sign broadcast tiles
        nc.vector.scalar_tensor_tensor(
            out=w3a_sb, in0=w3_sb, scalar=-1.0, in1=w3_sb, op0=ALU.mult, op1=ALU.max
        )
        nc.vector.tensor_mul(b2w_sb, b2_sb, w3a_sb)
        nc.vector.tensor_scalar(
            out=s_sb, in0=w3_sb, scalar1=0.0, scalar2=None, op0=ALU.is_ge
        )
        nc.vector.tensor_scalar(
            out=s_sb, in0=s_sb, scalar1=2.0, scalar2=-1.0, op0=ALU.mult, op1=ALU.add
        )
        nc.vector.memset(ones_h, 1.0)

    def emit_weight_load(wsrc, wtmp, wh, engine):
        # chunked fp32 stage + fp16 cast, so the first MLP layer can start
        # as soon as its first contraction chunk has landed
        for c in range(DC):
            nc.sync.dma_start(
                out=wtmp[:, c, :],
                in_=wsrc.rearrange("(c p) j -> p c j", p=128)[:, c, :],
            )
            if engine == "act":
                nc.scalar.activation(out=wh[:, c, :], in_=wtmp[:, c, :], func=AF.Copy)
            else:
                nc.vector.tensor_copy(wh[:, c, :], wtmp[:, c, :])

    n_iter = n_bk // PAIR

    def emit_load(it):
        # one tile per (pair, which) so each transpose chain only waits on
        # its own slice of the DMA traffic
        tiles = []
        for q in range(PAIR * 2):
            rq = raw.tile([128, D], F32, tag="raw", bufs=3 * PAIR * 2, name=f"raw_{it}_{q}")
            nc.sync.dma_start(
                out=rq, in_=x[it * PAIR + q // 2, q % 2]
            )
            tiles.append(rq)
        return tiles

    def emit_casts(it, raw_t):
        # fp16 copies of the raw tiles: fp16 transposes load their
        # stationary in 97ns vs fp32's ~140 (the transpose cadence is
        # LDWEIGHTS-bound).  Emitted FIFO-late so the DMA wait of a fresh
        # tile can never sit ahead of hot evacuations.
        tiles = []
        for q in range(PAIR * 2):
            rh = raw.tile([128, D], F16, tag="rawh", bufs=3 * PAIR * 2, name=f"rawh_{it}_{q}")
            if it == 0:
                # startup: ACT is idle while DVE casts w1
                nc.scalar.activation(out=rh, in_=raw_t[q], func=AF.Copy)
            else:
                nc.vector.tensor_copy(rh, raw_t[q])
            tiles.append(rh)
        return tiles

    def emit_transposes(it, raw_t):
        # XT: [d_part, chunk, col] fp16.  Per chunk: 4 transposes into one
        # [128, 512] PSUM tile, then a single wide cast-evacuation,
        # alternating DVE/ACT so neither engine throttles the PE.
        xt_t = xt.tile([128, DC, NCOL], F16, tag="xt", name=f"xt_{it}")
        for c in range(DC):
            tr_ps = p512.tile([128, NCOL], F16, tag="p512", name=f"tr_{it}_{c}")
            for q in range(PAIR * 2):
                nc.tensor.transpose(
                    tr_ps[:, q * 128 : (q + 1) * 128],
                    raw_t[q][:, c * 128 : (c + 1) * 128],
                    ident_h,
                )
            if it == 0 or c % 2 == 0:
                nc.vector.tensor_copy(xt_t[:, c, :], tr_ps)
            else:
                nc.scalar.activation(out=xt_t[:, c, :], in_=tr_ps, func=AF.Copy)
        return xt_t

    def emit_scores(it, xt_t):
        # scores + one-hot per pair; fp16 operands, exact fp32 accumulation
        onehots = []
        sall = p128.tile([128, PAIR * 128], F32, tag="p128", name=f"s_{it}")
        for p in range(PAIR):
            s_ps = sall[:, p * 128 : (p + 1) * 128]
            for c in range(DC):
                nc.tensor.matmul(
                    s_ps,
                    lhsT=xt_t[:, c, (2 * p) * 128 : (2 * p + 1) * 128],
                    rhs=xt_t[:, c, (2 * p + 1) * 128 : (2 * p + 2) * 128],
                    start=(c == 0),
                    stop=(c == DC - 1),
                )
            rm = small.tile([128, 1], F32, tag="rm", name=f"rm_{it}_{p}")
            nc.vector.reduce_max(rm, s_ps, axis=mybir.AxisListType.X)
            oh = oh2[(it * PAIR + p) % 4]
            nc.vector.tensor_scalar(
                out=oh[:, 128:256], in0=s_ps, scalar1=rm, scalar2=None,
                op0=ALU.is_equal,
            )
            onehots.append(oh)
        return onehots

    def emit_l1(it, src_t):
        # dst[j, col] = relu(sum_c W1[c,j].T @ src[c] + b1_j)
        dst_t = hp.tile([128, DC, NCOL], F16, tag="h", name=f"h_l1_{it}")
        for j in range(DC):
            mm = pmm.tile([128, NCOL], F32, tag="mm", name=f"mm_l1_{it}_{j}")
            for c in range(DC):
                nc.tensor.matmul(
                    mm,
                    lhsT=w1h[:, c, j * 128 : (j + 1) * 128],
                    rhs=src_t[:, c, :],
                    start=(c == 0),
                    stop=(c == DC - 1),
                )
            nc.scalar.activation(
                out=dst_t[:, j, :], in_=mm, func=AF.Relu, bias=b1_sb[:, j : j + 1]
            )
        return dst_t

    def emit_l2(it, src_t):
        # ap[j, col] = |w3_j| * relu(sum_c W2[c,j].T @ src[c] + b2_j)
        # (the |w3| scale rides the ACT evacuation for free)
        ap_t = hp.tile([128, DC, NCOL], F16, tag="h", name=f"h_l2_{it}")
        for j in range(DC):
            mm = pmm.tile([128, NCOL], F32, tag="mm", name=f"mm_l2_{it}_{j}")
            for c in range(DC):
                nc.tensor.matmul(
                    mm,
                    lhsT=w2h[:, c, j * 128 : (j + 1) * 128],
                    rhs=src_t[:, c, :],
                    start=(c == 0),
                    stop=(c == DC - 1),
                )
            nc.scalar.activation(
                out=ap_t[:, j, :], in_=mm, func=AF.Relu,
                bias=b2w_sb[:, j : j + 1], scale=w3a_sb[:, j : j + 1],
            )
        return ap_t

    def emit_achain(it, ap_t, f16_chain=False):
        # a[p, col] = sum_j sign(w3_j)[p] * ap_j[p, col] on DVE (fused
        # mult-accumulate); final step writes fp16 for the PE.  f16_chain
        # runs the intermediates in fp16 (2x DVE rate) -- used only for the
        # epilogue where the chain is exposed serially.
        idt = F16 if f16_chain else F32
        acc = scratch.tile([128, NCOL], idt, tag="af", bufs=2, name=f"a_{it}_0")
        nc.vector.tensor_scalar(
            out=acc, in0=ap_t[:, 0, :], scalar1=s_sb[:, 0:1], scalar2=None,
            op0=ALU.mult,
        )
        for c in range(1, DC):
            nxt = (
                scratch.tile([128, NCOL], F16, tag="ah", bufs=2, name=f"ah_{it}")
                if c == DC - 1
                else scratch.tile([128, NCOL], idt, tag="af", bufs=2, name=f"a_{it}_{c}")
            )
            nc.vector.scalar_tensor_tensor(
                out=nxt, in0=ap_t[:, c, :], scalar=s_sb[:, c : c + 1],
                in1=acc, op0=ALU.mult, op1=ALU.add,
            )
            acc = nxt
        return acc

    def emit_obc(it, a_h):
        # obc[l, col] = sum_p a[p, col] = f(col-row) - b3, reduced over
        # partitions AND broadcast to all 128 by the all-ones stationary
        obc = posm.tile([128, NCOL], F32, tag="obc", name=f"obc_{it}")
        nc.tensor.matmul(obc, lhsT=ones_h, rhs=a_h, start=True, stop=True)
        return obc

    def emit_tail(it, obc, onehots):
        # res[l] = obc[l, ctx_l] + sum_m onehot[l,m] * obc[l, ent_m] + 2*b3.
        # ctx and ent columns are adjacent in obc, and the mask tile is
        # [ident | onehot], so one fused mult + free-dim accumulate yields
        # rctx + rent at once.
        for p in range(PAIR):
            prod = scratch.tile([128, 256], F32, tag="prod", name=f"prod_{it}_{p}")
            rsum = small.tile([128, 1], F32, tag="rent", name=f"rsum_{it}_{p}")
            nc.vector.scalar_tensor_tensor(
                out=prod, in0=onehots[p], scalar=1.0,
                in1=obc[:, (2 * p) * 128 : (2 * p + 2) * 128],
                op0=ALU.mult, op1=ALU.mult, accum_out=rsum,
            )
            nc.vector.tensor_scalar(
                out=res_all[:, it * PAIR + p : it * PAIR + p + 1],
                in0=rsum, scalar1=b3x2, scalar2=None, op0=ALU.add,
            )

    # Two-stage software pipeline over iterations: stage A(i) = load/transpose/
    # scores/L1, stage B(i) = L2/obc/tail.  B(i-1) pieces are interleaved
    # into A(i) so the PE always has independent work while evacuations and
    # the DVE tail of the previous iteration drain.
    state = {}
    prev = None
    prev2 = None
    raw_next = emit_load(0)
    emit_const_loads()
    emit_weight_load(w1, wtmp1, w1h, "dve")
    rawh_next = emit_casts(0, raw_next)
    for it in range(n_iter):
        raw_t = rawh_next
        if it + 1 < n_iter:
            raw_next = emit_load(it + 1)
        if prev is not None:
            state[prev]["ap"] = emit_l2(prev, state[prev]["h1"])
        xt_t = emit_transposes(it, raw_t)
        # tail(i-2) here: its obc dependency resolved long ago, so it can't
        # head-of-line-block the DVE queue ahead of the transpose evacs
        if prev2 is not None:
            emit_tail(prev2, state[prev2]["obc"], state[prev2]["oh"])
            del state[prev2]
        if prev is not None:
            state[prev]["ah"] = emit_achain(prev, state[prev]["ap"])
        onehots = emit_scores(it, xt_t)
        if it == 0:
            emit_weight_load(w2, wtmp2, w2h, "dve")
        h1 = emit_l1(it, xt_t)
        if prev is not None:
            state[prev]["obc"] = emit_obc(prev, state[prev]["ah"])
        if it + 1 < n_iter:
            rawh_next = emit_casts(it + 1, raw_next)
        if it == 1:
            # sign-broadcast tiles for the epilogue obc; built here (not at
            # const-load time) so they never sit ahead of startup-critical
            # work in the DVE queue
            nc.vector.memset(zeros_sb, 0.0)
            for c in range(DC):
                nc.vector.tensor_scalar(
                    out=sgn_h[:, c, :], in0=zeros_sb, scalar1=s_sb[:, c : c + 1],
                    scalar2=None, op0=ALU.add,
                )
        state[it] = {"h1": h1, "oh": onehots}
        prev2 = prev
        prev = it
    # epilogue for the last two iterations
    if prev2 is not None:
        emit_tail(prev2, state[prev2]["obc"], state[prev2]["oh"])
        del state[prev2]
    ap_t = emit_l2(prev, state[prev]["h1"])
    obc = posm.tile([128, NCOL], F32, tag="obc", name="obc_epi")
    for j in range(DC):
        nc.tensor.matmul(
            obc, lhsT=sgn_h[:, j, :], rhs=ap_t[:, j, :],
            start=(j == 0), stop=(j == DC - 1),
        )
    emit_tail(prev, obc, state[prev]["oh"])

    # ---- store: transpose res_all [l_part, bk] on PE, contiguous DMA out ----
    res_ps = posm.tile([n_bk, 128], F32, tag="obc", name="res_ps")
    nc.tensor.transpose(res_ps, res_all, ident)
    res_T = small.tile([n_bk, 128], F32, tag="resT", name="res_T")
    nc.vector.tensor_copy(res_T, res_ps)
    nc.sync.dma_start(out=out[:, :], in_=res_T)


_NC_CACHE = {}


def _get_nc(n_bk):
    if n_bk not in _NC_CACHE:
        _NC_CACHE[n_bk] = build_kernel(n_bk)
    return _NC_CACHE[n_bk]


def run(inputs, trace=False):
    context = np.ascontiguousarray(np.asarray(inputs["context"], dtype=np.float32))
    xs = context.reshape(BK, 2, L, D)
    shared = {
        "w1": np.ascontiguousarray(np.asarray(inputs["W1"], dtype=np.float32)),
        "b1": np.ascontiguousarray(np.asarray(inputs["b1"], dtype=np.float32)),
        "w2": np.ascontiguousarray(np.asarray(inputs["W2"], dtype=np.float32)),
        "b2": np.ascontiguousarray(np.asarray(inputs["b2"], dtype=np.float32)),
        "w3": np.ascontiguousarray(np.asarray(inputs["W3"], dtype=np.float32)),
        "b3": np.ascontiguousarray(np.asarray(inputs["b3"], dtype=np.float32)),
    }
    in_maps = [
        {"x": np.ascontiguousarray(xs[c * BK_PER_CORE : (c + 1) * BK_PER_CORE]), **shared}
        for c in range(N_CORES)
    ]
    nc = _get_nc(BK_PER_CORE)
    res = run_bass_kernel_spmd(nc, in_maps, list(range(N_CORES)), trace=trace)
    outs = [m["out"] for m in res.results]
    full = np.concatenate(outs, axis=0).reshape(B, K, L).astype(np.float32)
    return full, res


def kernel(**inputs) -> np.ndarray:
    full, _ = run(inputs, trace=False)
    return full


# revision 32
# speedup vs baseline: 1.0035x; 1.0000x over previous
"""Trainium2 Bass kernel for nn_MlpwithSOMModule (retrieval_knn).

Reference computation, per (b, k) pair with L=128, D=768:
    ctx, ent = context[b,k,0], context[b,k,1]          # [L, D] each
    S        = ctx @ ent.T                             # [L, L]
    idx      = argmax_m S[l, m]
    best     = ent[idx]                                # [L, D]
    out[l]   = f(ctx[l]) + f(best[l])                  # f = 3-layer MLP -> scalar
Gather resolved as a one-hot weighted sum of scalars:
    out[l] = f(ctx[l]) + sum_m onehot[l,m] * f(ent[m]),  onehot = (S == rowmax(S))

All matmuls contract over D, so activations live in transposed layout
[d_partition, row_free]; raw inputs are transposed once on the PE and every
later layer produces its output already transposed.

Precision: everything fp16 (1 cy/row on the PE vs 4 for fp32).  Scores use
fp16 operands with exact fp32 PSUM accumulation; on the staged inputs this
flips 18 of 32768 argmax rows (rel_l2 1.1e-2 < 2e-2 gate) and the min fp16
top-2 score gap is 1.07e-4 -- far above HW summation-order noise, so the HW
flip count matches the numpy simulation (HW-verified: 1.106e-2, identical).
MLP in fp16 contributes 4.4e-4.

The third MLP layer (768->1, w3) is folded away entirely:
  - |w3_j| becomes the ACT scale of the L2 ReLU evacuation
    (ap_j = relu(z2*|w3_j| + |w3_j|*b2_j) = |w3_j| * relu(z2 + b2_j)),
  - sign(w3_j) rides the DVE sign-chain a = sum_j s_j*ap_j (fused
    scalar_tensor_tensor), and one ones.T @ a matmul both reduces over
    partitions and broadcasts f(row) to all 128 partitions,
  - b3 is added by the tail's fused scalar_tensor_tensor.

Transposes run in fp16 (raw tiles are pre-cast; fp16 LDWEIGHTS is 97ns vs
fp32's ~140 and the transpose cadence is LDWEIGHTS-bound, measured ~73ns
vs 125), grouped 4-per-PSUM-tile with one wide 2-byte evacuation per chunk
alternating DVE/ACT.  Scheduling is FIFO-aware throughout: weight casts
and pre-casts are emitted so a DMA wait can never sit ahead of a hot
evacuation in an engine queue (head-of-line blocking there showed up as
0.7-5.6us/iteration of PE stall), and the tail is deferred one extra
pipeline stage so its obc dependency is resolved before it reaches the
DVE queue head.  HW: 319us vs the 451us f32r baseline (PE ~93% busy,
MLP matmuls at the 213ns N=512 streaming floor).

Sharding: data-parallel over the 256 (b,k) pairs -> 32 per NeuronCore,
weights replicated.  Two pairs per inner iteration (moving dim 512).
"""

from contextlib import ExitStack

import numpy as np

import concourse.bacc as bacc
import concourse.mybir as mybir
import concourse.tile as tile
from concourse.bass_utils import run_bass_kernel_spmd
from concourse.masks import make_identity

B, K, L, D = 4, 64, 128, 768
N_CORES = 8
BK = B * K                      # 256 (b,k) pairs total
BK_PER_CORE = BK // N_CORES     # 32
PAIR = 2                        # pairs per inner iteration (moving dim 512)
DC = D // 128                   # 6 contraction chunks
NCOL = PAIR * 2 * 128           # 512 columns per iteration

F32 = mybir.dt.float32
F16 = mybir.dt.float16


def build_kernel(n_bk: int = BK_PER_CORE):
    assert n_bk % PAIR == 0
    nc = bacc.Bacc("TRN2", target_bir_lowering=False)

    x = nc.declare_dram_parameter("x", [n_bk, 2, L, D], F32, isOutput=False)
    w1 = nc.declare_dram_parameter("w1", [D, D], F32, isOutput=False)
    b1 = nc.declare_dram_parameter("b1", [D], F32, isOutput=False)
    w2 = nc.declare_dram_parameter("w2", [D, D], F32, isOutput=False)
    b2 = nc.declare_dram_parameter("b2", [D], F32, isOutput=False)
    w3 = nc.declare_dram_parameter("w3", [D, 1], F32, isOutput=False)
    b3 = nc.declare_dram_parameter("b3", [1], F32, isOutput=False)
    out = nc.declare_dram_parameter("out", [n_bk, L], F32, isOutput=True)

    with tile.TileContext(nc) as tc:
        with ExitStack() as ctx:
            _emit(ctx, tc, n_bk, x, w1, b1, w2, b2, w3, b3, out)
    nc.compile()
    return nc


def _emit(ctx, tc, n_bk, x, w1, b1, w2, b2, w3, b3, out):
    nc = tc.nc
    AF = mybir.ActivationFunctionType
    ALU = mybir.AluOpType

    consts = ctx.enter_context(tc.tile_pool(name="consts", bufs=1))
    raw = ctx.enter_context(tc.tile_pool(name="raw", bufs=1))
    xt = ctx.enter_context(tc.tile_pool(name="xt", bufs=3))
    hp = ctx.enter_context(tc.tile_pool(name="hp", bufs=3))
    small = ctx.enter_context(tc.tile_pool(name="small", bufs=4))
    scratch = ctx.enter_context(tc.tile_pool(name="scratch", bufs=4))
    pmm = ctx.enter_context(tc.tile_pool(name="pmm", bufs=2, space="PSUM"))
    p512 = ctx.enter_context(tc.tile_pool(name="p512", bufs=4, space="PSUM"))
    p128 = ctx.enter_context(tc.tile_pool(name="p128", bufs=1, space="PSUM"))
    posm = ctx.enter_context(tc.tile_pool(name="posm", bufs=1, space="PSUM"))

    # ---- constant tiles (DMAs emitted later, after the first raw loads) ----
    b1_sb = consts.tile([128, DC], F32)
    b2_sb = consts.tile([128, DC], F32)
    b3_sb = consts.tile([1, 1], F32)
    w3_sb = consts.tile([128, DC], F32)
    w1h = consts.tile([128, DC, D], F16)
    w2h = consts.tile([128, DC, D], F16)
    wtmp1 = consts.tile([128, DC, D], F32)
    wtmp2 = consts.tile([128, DC, D], F32)
    w3a_sb = consts.tile([128, DC], F32)       # |w3|
    b2w_sb = consts.tile([128, DC], F32)       # b2 * |w3|
    s_sb = consts.tile([128, DC], F32)         # sign(w3) in {+1,-1}
    ones_h = consts.tile([128, 128], F16)
    sgn_h = consts.tile([128, DC, 128], F16)   # sign(w3) broadcast along free
    zeros_sb = consts.tile([128, 128], F32)
    b3x2 = consts.tile([128, 1], F32)          # 2*b3 on every partition

    ident = consts.tile([128, 128], F32)
    make_identity(nc, ident)
    ident_h = consts.tile([128, 128], F16)
    nc.vector.tensor_copy(ident_h, ident)
    # HAM warm-up: ~3.4us of dummy transposes inside the first-DMA shadow so
    # the clock gate is at 2.4GHz when the real startup transposes begin
    # (outputs are never read; the pool reuses the bank afterwards)
    warm = p512.tile([128, NCOL], F16, tag="p512", name="warm")
    for i in range(16):
        nc.tensor.transpose(
            warm[:, (i % 4) * 128 : (i % 4 + 1) * 128], ident_h, ident_h
        )
    oh2 = [consts.tile([128, 256], F32, name=f"oh2_{i}") for i in range(4)]
    for t in oh2:
        nc.vector.tensor_copy(t[:, 0:128], ident)

    res_all = consts.tile([128, n_bk], F32)

    def emit_const_loads():
        # small consts; emitted after the first raw-tile DMAs so iteration
        # 0's transposes aren't queued behind them
        nc.sync.dma_start(out=b1_sb, in_=b1.rearrange("(c p) -> p c", p=128))
        nc.sync.dma_start(out=b2_sb, in_=b2.rearrange("(c p) -> p c", p=128))
        nc.sync.dma_start(out=b3_sb, in_=b3[:].unsqueeze(0))
        nc.sync.dma_start(out=w3_sb, in_=w3.rearrange("(c p) one -> p (c one)", p=128))
        nc.gpsimd.partition_broadcast(b3x2, b3_sb, channels=128)
        nc.vector.tensor_scalar_mul(b3x2, b3x2, 2.0)
        # |w3|, b2*|w3|, sign(w3), and the sign broadcast tiles
        nc.vector.scalar_tensor_tensor(
            out=w3a_sb, in0=w3_sb, scalar=-1.0, in1=w3_sb, op0=ALU.mult, op1=ALU.max
        )
        nc.vector.tensor_mul(b2w_sb, b2_sb, w3a_sb)
        nc.vector.tensor_scalar(
            out=s_sb, in0=w3_sb, scalar1=0.0, scalar2=None, op0=ALU.is_ge
        )
        nc.vector.tensor_scalar(
            out=s_sb, in0=s_sb, scalar1=2.0, scalar2=-1.0, op0=ALU.mult, op1=ALU.add
        )
        nc.vector.memset(ones_h, 1.0)

    def emit_weight_load(wsrc, wtmp, wh, engine):
        # chunked fp32 stage + fp16 cast, so the first MLP layer can start
        # as soon as its first contraction chunk has landed
        for c in range(DC):
            nc.sync.dma_start(
                out=wtmp[:, c, :],
                in_=wsrc.rearrange("(c p) j -> p c j", p=128)[:, c, :],
            )
            if engine == "act":
                nc.scalar.activation(out=wh[:, c, :], in_=wtmp[:, c, :], func=AF.Copy)
            else:
                nc.vector.tensor_copy(wh[:, c, :], wtmp[:, c, :])

    n_iter = n_bk // PAIR

    def emit_load(it):
        # one tile per (pair, which) so each transpose chain only waits on
        # its own slice of the DMA traffic
        tiles = []
        for q in range(PAIR * 2):
            rq = raw.tile([128, D], F32, tag="raw", bufs=3 * PAIR * 2, name=f"raw_{it}_{q}")
            nc.sync.dma_start(
                out=rq, in_=x[it * PAIR + q // 2, q % 2]
            )
            tiles.append(rq)
        return tiles

    def emit_casts(it, raw_t):
        # fp16 copies of the raw tiles: fp16 transposes load their
        # stationary in 97ns vs fp32's ~140 (the transpose cadence is
        # LDWEIGHTS-bound).  Emitted FIFO-late so the DMA wait of a fresh
        # tile can never sit ahead of hot evacuations.
        tiles = []
        for q in range(PAIR * 2):
            rh = raw.tile([128, D], F16, tag="rawh", bufs=3 * PAIR * 2, name=f"rawh_{it}_{q}")
            if it == 0:
                # startup: ACT is idle while DVE casts w1
                nc.scalar.activation(out=rh, in_=raw_t[q], func=AF.Copy)
            else:
                nc.vector.tensor_copy(rh, raw_t[q])
            tiles.append(rh)
        return tiles

    def emit_transposes(it, raw_t):
        # XT: [d_part, chunk, col] fp16.  Per chunk: 4 transposes into one
        # [128, 512] PSUM tile, then a single wide cast-evacuation,
        # alternating DVE/ACT so neither engine throttles the PE.
        xt_t = xt.tile([128, DC, NCOL], F16, tag="xt", name=f"xt_{it}")
        for c in range(DC):
            tr_ps = p512.tile([128, NCOL], F16, tag="p512", name=f"tr_{it}_{c}")
            for q in range(PAIR * 2):
                nc.tensor.transpose(
                    tr_ps[:, q * 128 : (q + 1) * 128],
                    raw_t[q][:, c * 128 : (c + 1) * 128],
                    ident_h,
                )
            if it == 0 or c % 2 == 0:
                nc.vector.tensor_copy(xt_t[:, c, :], tr_ps)
            else:
                nc.scalar.activation(out=xt_t[:, c, :], in_=tr_ps, func=AF.Copy)
        return xt_t

    def emit_scores(it, xt_t):
        # scores + one-hot per pair; fp16 operands, exact fp32 accumulation
        onehots = []
        sall = p128.tile([128, PAIR * 128], F32, tag="p128", name=f"s_{it}")
        for p in range(PAIR):
            s_ps = sall[:, p * 128 : (p + 1) * 128]
            for c in range(DC):
                nc.tensor.matmul(
                    s_ps,
                    lhsT=xt_t[:, c, (2 * p) * 128 : (2 * p + 1) * 128],
                    rhs=xt_t[:, c, (2 * p + 1) * 128 : (2 * p + 2) * 128],
                    start=(c == 0),
                    stop=(c == DC - 1),
                )
            rm = small.tile([128, 1], F32, tag="rm", name=f"rm_{it}_{p}")
            nc.vector.reduce_max(rm, s_ps, axis=mybir.AxisListType.X)
            oh = oh2[(it * PAIR + p) % 4]
            nc.vector.tensor_scalar(
                out=oh[:, 128:256], in0=s_ps, scalar1=rm, scalar2=None,
                op0=ALU.is_equal,
            )
            onehots.append(oh)
        return onehots

    def emit_l1(it, src_t):
        # dst[j, col] = relu(sum_c W1[c,j].T @ src[c] + b1_j)
        dst_t = hp.tile([128, DC, NCOL], F16, tag="h", name=f"h_l1_{it}")
        for j in range(DC):
            mm = pmm.tile([128, NCOL], F32, tag="mm", name=f"mm_l1_{it}_{j}")
            for c in range(DC):
                nc.tensor.matmul(
                    mm,
                    lhsT=w1h[:, c, j * 128 : (j + 1) * 128],
                    rhs=src_t[:, c, :],
                    start=(c == 0),
                    stop=(c == DC - 1),
                )
            nc.scalar.activation(
                out=dst_t[:, j, :], in_=mm, func=AF.Relu, bias=b1_sb[:, j : j + 1]
            )
        return dst_t

    def emit_l2(it, src_t):
        # ap[j, col] = |w3_j| * relu(sum_c W2[c,j].T @ src[c] + b2_j)
        # (the |w3| scale rides the ACT evacuation for free)
        ap_t = hp.tile([128, DC, NCOL], F16, tag="h", name=f"h_l2_{it}")
        for j in range(DC):
            mm = pmm.tile([128, NCOL], F32, tag="mm", name=f"mm_l2_{it}_{j}")
            for c in range(DC):
                nc.tensor.matmul(
                    mm,
                    lhsT=w2h[:, c, j * 128 : (j + 1) * 128],
                    rhs=src_t[:, c, :],
                    start=(c == 0),
                    stop=(c == DC - 1),
                )
            nc.scalar.activation(
                out=ap_t[:, j, :], in_=mm, func=AF.Relu,
                bias=b2w_sb[:, j : j + 1], scale=w3a_sb[:, j : j + 1],
            )
        return ap_t

    def emit_achain(it, ap_t, f16_chain=False):
        # a[p, col] = sum_j sign(w3_j)[p] * ap_j[p, col] on DVE (fused
        # mult-accumulate); final step writes fp16 for the PE.  f16_chain
        # runs the intermediates in fp16 (2x DVE rate) -- used only for the
        # epilogue where the chain is exposed serially.
        idt = F16 if f16_chain else F32
        acc = scratch.tile([128, NCOL], idt, tag="af", bufs=2, name=f"a_{it}_0")
        nc.vector.tensor_scalar(
            out=acc, in0=ap_t[:, 0, :], scalar1=s_sb[:, 0:1], scalar2=None,
            op0=ALU.mult,
        )
        for c in range(1, DC):
            nxt = (
                scratch.tile([128, NCOL], F16, tag="ah", bufs=2, name=f"ah_{it}")
                if c == DC - 1
                else scratch.tile([128, NCOL], idt, tag="af", bufs=2, name=f"a_{it}_{c}")
            )
            nc.vector.scalar_tensor_tensor(
                out=nxt, in0=ap_t[:, c, :], scalar=s_sb[:, c : c + 1],
                in1=acc, op0=ALU.mult, op1=ALU.add,
            )
            acc = nxt
        return acc

    def emit_obc(it, a_h):
        # obc[l, col] = sum_p a[p, col] = f(col-row) - b3, reduced over
        # partitions AND broadcast to all 128 by the all-ones stationary
        obc = posm.tile([128, NCOL], F32, tag="obc", name=f"obc_{it}")
        nc.tensor.matmul(obc, lhsT=ones_h, rhs=a_h, start=True, stop=True)
        return obc

    def emit_tail(it, obc, onehots):
        # res[l] = obc[l, ctx_l] + sum_m onehot[l,m] * obc[l, ent_m] + 2*b3.
        # ctx and ent columns are adjacent in obc, and the mask tile is
        # [ident | onehot], so one fused mult + free-dim accumulate yields
        # rctx + rent at once.
        for p in range(PAIR):
            prod = scratch.tile([128, 256], F32, tag="prod", name=f"prod_{it}_{p}")
            rsum = small.tile([128, 1], F32, tag="rent", name=f"rsum_{it}_{p}")
            nc.vector.scalar_tensor_tensor(
                out=prod, in0=onehots[p], scalar=1.0,
                in1=obc[:, (2 * p) * 128 : (2 * p + 2) * 128],
                op0=ALU.mult, op1=ALU.mult, accum_out=rsum,
            )
            nc.vector.tensor_scalar(
                out=res_all[:, it * PAIR + p : it * PAIR + p + 1],
                in0=rsum, scalar1=b3x2, scalar2=None, op0=ALU.add,
            )

    # Two-stage software pipeline over iterations: stage A(i) = load/transpose/
    # scores/L1, stage B(i) = L2/obc/tail.  B(i-1) pieces are interleaved
    # into A(i) so the PE always has independent work while evacuations and
    # the DVE tail of the previous iteration drain.
    state = {}
    prev = None
    prev2 = None
    raw_next = emit_load(0)
    emit_const_loads()
    emit_weight_load(w1, wtmp1, w1h, "dve")
    rawh_next = emit_casts(0, raw_next)
    for it in range(n_iter):
        raw_t = rawh_next
        if it + 1 < n_iter:
            raw_next = emit_load(it + 1)
        if prev is not None:
            state[prev]["ap"] = emit_l2(prev, state[prev]["h1"])
        xt_t = emit_transposes(it, raw_t)
        # tail(i-2) here: its obc dependency resolved long ago, so it can't
        # head-of-line-block the DVE queue ahead of the transpose evacs
        if prev2 is not None:
            emit_tail(prev2, state[prev2]["obc"], state[prev2]["oh"])
            del state[prev2]
        if prev is not None:
            state[prev]["ah"] = emit_achain(prev, state[prev]["ap"])
        onehots = emit_scores(it, xt_t)
        if it == 0:
            emit_weight_load(w2, wtmp2, w2h, "dve")
        h1 = emit_l1(it, xt_t)
        if prev is not None:
            state[prev]["obc"] = emit_obc(prev, state[prev]["ah"])
        if it + 1 < n_iter:
            rawh_next = emit_casts(it + 1, raw_next)
        if it == 1:
            # sign-broadcast tiles for the epilogue obc; built here (not at
            # const-load time) so they never sit ahead of startup-critical
            # work in the DVE queue
            nc.vector.memset(zeros_sb, 0.0)
            for c in range(DC):
                nc.vector.tensor_scalar(
                    out=sgn_h[:, c, :], in0=zeros_sb, scalar1=s_sb[:, c : c + 1],
                    scalar2=None, op0=ALU.add,
                )
        state[it] = {"h1": h1, "oh": onehots}
        prev2 = prev
        prev = it
    # epilogue for the last two iterations
    if prev2 is not None:
        emit_tail(prev2, state[prev2]["obc"], state[prev2]["oh"])
        del state[prev2]
    ap_t = emit_l2(prev, state[prev]["h1"])
    obc = posm.tile([128, NCOL], F32, tag="obc", name="obc_epi")
    for j in range(DC):
        nc.tensor.matmul(
            obc, lhsT=sgn_h[:, j, :], rhs=ap_t[:, j, :],
            start=(j == 0), stop=(j == DC - 1),
        )
    emit_tail(prev, obc, state[prev]["oh"])

    # ---- store: transpose res_all [l_part, bk] on PE, contiguous DMA out ----
    res_ps = posm.tile([n_bk, 128], F32, tag="obc", name="res_ps")
    nc.tensor.transpose(res_ps, res_all, ident)
    res_T = small.tile([n_bk, 128], F32, tag="resT", name="res_T")
    nc.vector.tensor_copy(res_T, res_ps)
    nc.sync.dma_start(out=out[:, :], in_=res_T)


_NC_CACHE = {}


def _get_nc(n_bk):
    if n_bk not in _NC_CACHE:
        _NC_CACHE[n_bk] = build_kernel(n_bk)
    return _NC_CACHE[n_bk]


def run(inputs, trace=False):
    context = np.ascontiguousarray(np.asarray(inputs["context"], dtype=np.float32))
    xs = context.reshape(BK, 2, L, D)
    shared = {
        "w1": np.ascontiguousarray(np.asarray(inputs["W1"], dtype=np.float32)),
        "b1": np.ascontiguousarray(np.asarray(inputs["b1"], dtype=np.float32)),
        "w2": np.ascontiguousarray(np.asarray(inputs["W2"], dtype=np.float32)),
        "b2": np.ascontiguousarray(np.asarray(inputs["b2"], dtype=np.float32)),
        "w3": np.ascontiguousarray(np.asarray(inputs["W3"], dtype=np.float32)),
        "b3": np.ascontiguousarray(np.asarray(inputs["b3"], dtype=np.float32)),
    }
    in_maps = [
        {"x": np.ascontiguousarray(xs[c * BK_PER_CORE : (c + 1) * BK_PER_CORE]), **shared}
        for c in range(N_CORES)
    ]
    nc = _get_nc(BK_PER_CORE)
    res = run_bass_kernel_spmd(nc, in_maps, list(range(N_CORES)), trace=trace)
    outs = [m["out"] for m in res.results]
    full = np.concatenate(outs, axis=0).reshape(B, K, L).astype(np.float32)
    return full, res


def kernel(**inputs) -> np.ndarray:
    full, _ = run(inputs, trace=False)
    return full
